# revision 12
# baseline (speedup 1.0000x reference)
"""Trainium2 Bass kernel for nn_Encoder — v11 "v5 + asymmetric caps (512/488) + num_idxs=1000 tail skip".

Math (exact, up to float reordering):
  S[n] = (sum_{e: dst=n} xg[src_e]) + xg[n],   xg[m] = dis_m * xext_m
  x1[n] = relu(dis_n * c_n * (S[n] @ W1ext))   (b1 == 0 path; c,dis > 0)
  out = (1/N) * (sum_n x1[n]) @ W2 + b2        (layer 2 + mean collapsed)

v2 removes the baseline's dense phase 1 (y = x@W1 table) entirely: edges
gather raw (dis-scaled) feature rows xg directly from DRAM, self-loops are
ordinary edges in the stream, aggregation happens in 124-dim feature space
via one-hot matmuls into PSUM, and W1 is applied per dst tile after a
PE-transpose. Gathers run on all 4 SWDGE queues (one per src chunk) so the
4 Q7 pairs generate descriptors concurrently.

Sharding: dst nodes (and their incoming edges) split across 8 cores; xg
table replicated; per-core [128, 2*H] partial accumulations summed on host.
"""

import sys, os, types
sys.path.insert(0, "/opt/trn_rl_repo")

# antenv.axon_hooks shim (image's antenv stub lacks it); needed for NTFF trace.
if "antenv.axon_hooks" not in sys.modules:
    _hook = [None]
    _m = types.ModuleType("antenv.axon_hooks")
    _m.set_axon_ntff_profile_hook = lambda h: _hook.__setitem__(0, h)
    _m.get_axon_ntff_profile_hook = lambda: _hook[0]
    sys.modules["antenv.axon_hooks"] = _m
    try:
        import antenv
        antenv.axon_hooks = _m
        from trn_agent_boot.trn_boot import _ntff_profile_via_ctypes
        _m.set_axon_ntff_profile_hook(
            _ntff_profile_via_ctypes("/opt/axon/libaxon_pjrt.so"))
    except Exception:
        pass

import numpy as np
from contextlib import ExitStack
from dataclasses import dataclass

import concourse.bacc as bacc
import concourse.bass as bass
import concourse.mybir as mybir
import concourse.tile as tile
from concourse.bass_utils import run_bass_kernel_spmd
from concourse.library_config import mlp

P = 128
H = 128
F_IN = 116
FEXT = F_IN + 8          # 124 features (node + one-hot type), padded to 128
FPAD = 128
B = 2
XW = B * FPAD            # 256: xg row elements (both batches, padded)
YW = B * H               # 256: output row elements


@dataclass(frozen=True)
class Cfg:
    n: int = 100000      # nodes
    ncores: int = 8
    tiles: int = 104     # dst tiles per core (128 slots each)
    chunks: int = 16     # 128-edge chunks per tile (sum over 4 src chunks)
    group: int = 2       # tiles per gather-call group (= psum tiles in flight)
    nsc: int = 4         # src chunks (int16 gather index reach)

    @property
    def ndst(self):
        return self.n // self.ncores

    @property
    def srcchunk(self):
        return -(-self.n // self.nsc)

    @property
    def rot(self):        # rot[r][s]: chunks of tile (t%group==r) in src chunk s
        base, extra = divmod(self.chunks, self.nsc)
        return [[base + (1 if (s - r) % self.nsc < extra else 0)
                 for s in range(self.nsc)] for r in range(self.nsc)]

    @property
    def ngroups(self):
        return self.tiles // self.group

    @property
    def call_chunks(self):
        return sum(self.rot[r][0] for r in range(self.group))

    @property
    def call_idx(self):
        return self.call_chunks * P

    @property
    def ncalls(self):
        return self.ngroups * self.nsc

    @property
    def idxcols(self):
        return self.ncalls * (self.call_idx // 16)

    @property
    def nchunks_total(self):
        return self.tiles * self.chunks


CFG = Cfg()

f32 = mybir.dt.float32
f16 = mybir.dt.float16
f8 = mybir.dt.float8e4
i16 = mybir.dt.int16


def _build_program(cfg: Cfg, has_b1: bool):
    nc = bacc.Bacc("TRN2", num_swdge_queues=4)
    npad2 = cfg.nsc * cfg.srcchunk
    xg = nc.dram_tensor("xg", [npad2, XW], f8, kind="ExternalInput")
    xgo = nc.dram_tensor("xgo", [cfg.tiles * P, XW], f16, kind="ExternalInput")
    w1e = nc.dram_tensor("w1e", [FPAD, H], f16, kind="ExternalInput")
    idxt = nc.dram_tensor("idxt", [P, cfg.idxcols], i16, kind="ExternalInput")
    dlt = nc.dram_tensor("dlt", [P, cfg.nchunks_total], f16, kind="ExternalInput")
    dcq = nc.dram_tensor("dcq", [P, cfg.tiles], f32, kind="ExternalInput")
    iot = nc.dram_tensor("iot", [P, 4 * P], f16, kind="ExternalInput")
    idn = nc.dram_tensor("idn", [P, P], f16, kind="ExternalInput")
    if has_b1:
        disc = nc.dram_tensor("disc", [P, cfg.tiles], f32, kind="ExternalInput")
        cct = nc.dram_tensor("cct", [P, cfg.tiles], f32, kind="ExternalInput")
        b1b = nc.dram_tensor("b1b", [P, YW], f32, kind="ExternalInput")
    accd = nc.dram_tensor("acc", [P, YW], f32, kind="ExternalOutput")

    rotpre = [[sum(cfg.rot[i][s] for i in range(r)) for s in range(cfg.nsc)]
              for r in range(cfg.group)]

    with tile.TileContext(nc) as tc:
        nc.gpsimd.load_library(mlp)
        with (
            tc.tile_pool(name="const", bufs=1) as cpool,
            tc.tile_pool(name="gat", bufs=10) as gpool,
            tc.tile_pool(name="xop", bufs=4) as xopool,
            tc.tile_pool(name="oh", bufs=10) as ohpool,
            tc.tile_pool(name="agg", bufs=4) as apool,
            tc.tile_pool(name="x1c", bufs=4) as xpool,
            tc.tile_pool(name="psag", bufs=6, space="PSUM") as psag,
            tc.tile_pool(name="ps2", bufs=2, space="PSUM") as ps2,
            # psag 6 + ps2 2 = 8 PSUM banks
            ExitStack() as ctx,
        ):
            # constants / small preloads
            w1_sb = cpool.tile([FPAD, H], f16, tag="w1")
            nc.sync.dma_start(w1_sb[:], w1e[:])
            iota_sb = cpool.tile([P, 4, P], f16, tag="iota")
            nc.sync.dma_start(iota_sb[:], iot[:])
            iden_sb = cpool.tile([P, P], f16, tag="iden")
            nc.sync.dma_start(iden_sb[:], idn[:])
            dl_sb = cpool.tile([P, cfg.nchunks_total], f16, tag="dl")
            nc.sync.dma_start(dl_sb[:], dlt[:])
            dcq_sb = cpool.tile([P, cfg.tiles], f32, tag="dcq")
            nc.sync.dma_start(dcq_sb[:], dcq[:])
            if has_b1:
                disc_sb = cpool.tile([P, cfg.tiles], f32, tag="disc")
                nc.sync.dma_start(disc_sb[:], disc[:])
                cc_sb = cpool.tile([P, cfg.tiles], f32, tag="cc")
                nc.sync.dma_start(cc_sb[:], cct[:])
                b1_sb = cpool.tile([P, YW], f32, tag="b1b")
                nc.sync.dma_start(b1_sb[:], b1b[:])
            acc_sb = cpool.tile([P, YW], f32, tag="acc")
            nc.vector.memset(acc_sb[:], 0)

            ic_g = cfg.nsc * (cfg.call_idx // 16)  # idx cols per group
            for g in range(cfg.ngroups):
                # flipped agg psum: pst[ti][f, b, slot] = S^T per batch half
                pst = [psag.tile([P, B, P], f32, tag="psag", name=f"pst{g}_{i}")
                       for i in range(cfg.group)]
                # one idx DMA for the whole group's 4 gather calls
                idx_sb = gpool.tile([P, ic_g], i16, tag="idx")
                nc.sync.dma_start(
                    idx_sb[:], idxt[:, g * ic_g:(g + 1) * ic_g])
                # self-loop rows double as the psum-start matmuls:
                # pst[ti][:, b, :] = xo_b^T  (+= one-hot aggregation after)
                start_mm = [None] * cfg.group
                for ti in range(cfg.group):
                    t = g * cfg.group + ti
                    xo = xopool.tile([P, XW], f16, tag=f"xo{ti}")
                    nc.sync.dma_start(xo[:], xgo[t * P:(t + 1) * P, :])
                    sm0 = nc.tensor.matmul(
                        pst[ti][:, 0, :], lhsT=xo[:, 0:FPAD],
                        rhs=iden_sb[:], start=True, stop=False)
                    sm1 = nc.tensor.matmul(
                        pst[ti][:, 1, :], lhsT=xo[:, FPAD:2 * FPAD],
                        rhs=iden_sb[:], start=False, stop=False)
                    bass._add_dep_helper(
                        sm1.ins, sm0.ins, sync=False,
                        reason="self b1 after psum start")
                    start_mm[ti] = sm0
                for s in range(cfg.nsc):
                    call = g * cfg.nsc + s
                    gt = gpool.tile([P, cfg.call_chunks, XW], f8, tag="gt")
                    r0 = s * cfg.srcchunk
                    n_call = cfg.call_idx if call < 10 else cfg.call_idx - 24
                    nc.gpsimd.dma_gather(
                        gt[:], xg[r0:r0 + cfg.srcchunk, :],
                        idx_sb[:, s * (cfg.call_idx // 16):
                               (s + 1) * (cfg.call_idx // 16)],
                        n_call, n_call, XW, queue_num=s)
                    for ti in range(cfg.group):
                        k = cfg.rot[ti][s]
                        off = rotpre[ti][s]
                        assert k == 4
                        gcol = call * cfg.call_chunks + off
                        # fp8 one-hot quad for chunks (off .. off+3)
                        oh = ohpool.tile([P, 4, P], f8, tag="oh")
                        nc.vector.tensor_tensor(
                            out=oh[:],
                            in0=dl_sb[:, gcol:gcol + 4].to_broadcast(
                                [P, 4, P]),
                            in1=iota_sb[:],
                            op=mybir.AluOpType.is_equal)
                        # flipped DoubleRow per batch:
                        # pst[f, b, slot] += sum_i gt[:, c+i, bF].T @ oh[:, i]
                        for j2 in range(k // 2):
                            c0 = off + 2 * j2
                            for b in range(B):
                                gts = gt[:, c0:c0 + 2, b * FPAD:(b + 1) * FPAD]
                                mm = nc.tensor.matmul(
                                    pst[ti][:, b, :], lhsT=gts,
                                    rhs=oh[:, 2 * j2:2 * j2 + 2, :],
                                    start=False,
                                    stop=(s == cfg.nsc - 1 and
                                          j2 == k // 2 - 1 and b == B - 1),
                                    perf_mode=mybir.MatmulPerfMode.DoubleRow)
                                bass._add_dep_helper(
                                    mm.ins, start_mm[ti].ins, sync=False,
                                    reason="accum after psum start")
                for ti in range(cfg.group):
                    t = g * cfg.group + ti
                    # aggT [f, b, slot] f16 <- flipped agg psum (no transpose)
                    aggT = apool.tile([P, B, P], f16, tag="aggT")
                    nc.scalar.activation(
                        out=aggT[:], in_=pst[ti][:],
                        func=mybir.ActivationFunctionType.Copy)
                    # W1 application: psum2[slot, b*H:(b+1)*H] = aggT_b^T @ W1.
                    # Chained start/stop: the first mm's start zero-fills the
                    # whole bank, second mm accumulates into its (zeroed) half.
                    psum2 = ps2.tile([P, YW], f32, tag="ps2")
                    mm0 = nc.tensor.matmul(
                        psum2[:, 0:H], lhsT=aggT[:, 0, :],
                        rhs=w1_sb[:], start=True, stop=False)
                    mm1 = nc.tensor.matmul(
                        psum2[:, H:2 * H], lhsT=aggT[:, 1, :],
                        rhs=w1_sb[:], start=False, stop=True)
                    bass._add_dep_helper(
                        mm1.ins, mm0.ins, sync=False,
                        reason="second half after psum2 start")
                    x1c = xpool.tile([P, YW], f32, tag="x1c")
                    if not has_b1:
                        nc.scalar.activation(
                            out=x1c[:], in_=psum2[:],
                            func=mybir.ActivationFunctionType.Relu,
                            bias=0.0, scale=dcq_sb[:, t:t + 1])
                    else:
                        t1 = xpool.tile([P, YW], f32, tag="t1")
                        nc.vector.tensor_scalar(
                            out=t1[:], in0=psum2[:],
                            scalar1=disc_sb[:, t:t + 1], scalar2=None,
                            op0=mybir.AluOpType.mult)
                        nc.vector.tensor_tensor(
                            out=t1[:], in0=t1[:], in1=b1_sb[:],
                            op=mybir.AluOpType.add)
                        nc.scalar.activation(
                            out=t1[:], in_=t1[:],
                            func=mybir.ActivationFunctionType.Relu)
                        nc.vector.tensor_scalar(
                            out=x1c[:], in0=t1[:],
                            scalar1=cc_sb[:, t:t + 1], scalar2=None,
                            op0=mybir.AluOpType.mult)
                    nc.vector.tensor_tensor(
                        out=acc_sb[:], in0=acc_sb[:], in1=x1c[:],
                        op=mybir.AluOpType.add)

            nc.sync.dma_start(accd[:], acc_sb[:])

    nc.compile()
    return nc


_PROG_CACHE = {}


def _get_program(cfg: Cfg, has_b1: bool):
    key = (cfg, has_b1)
    if key not in _PROG_CACHE:
        _PROG_CACHE[key] = _build_program(cfg, has_b1)
    return _PROG_CACHE[key]


def _pack_core(cfg: Cfg, core, src, dst, dis_c, n_nodes):
    """Bin-pack this core's dst nodes into tiles; build gather/dstloc/dcq data.

    (self edges are handled by the xgo identity matmul, not the gather)
    Returns (idx_w [128, idxcols] i16, dl_w [128, nchunks] f16,
             dcq_w [128, tiles] f32, tile_of, slot_of)."""
    n0 = core * cfg.ndst
    sel = (dst >= n0) & (dst < n0 + cfg.ndst)
    es = src[sel]
    ed = dst[sel]
    dl = ed - n0                       # local dst id
    sc = es // cfg.srcchunk            # src chunk of each edge

    cnt = np.bincount(dl * cfg.nsc + sc, minlength=cfg.ndst * cfg.nsc)
    cnt = cnt.reshape(cfg.ndst, cfg.nsc)

    rot = np.array(cfg.rot, dtype=np.int64)          # [group, nsc]
    caps = (rot[np.arange(cfg.tiles) % cfg.group] * P).copy()  # [tiles, nsc]
    # odd (second-in-group) tiles cap at 488: their region ends the call, so
    # every call has >=24 trailing pad rows that num_idxs=1000 skips
    caps[1::2, :] -= 24
    for s in range(cfg.nsc):
        assert cnt[:, s].sum() <= caps[:, s].sum(), \
            f"core {core}: src chunk {s} demand exceeds capacity"

    order = np.argsort(-cnt.sum(1), kind="stable")
    slots_used = np.zeros(cfg.tiles, dtype=np.int64)
    tile_of = np.full(cfg.ndst, -1, dtype=np.int64)
    slot_of = np.full(cfg.ndst, -1, dtype=np.int64)
    for nloc in order:
        need = cnt[nloc]
        ok = (caps >= need).all(axis=1) & (slots_used < P)
        if not ok.any():
            raise RuntimeError(f"core {core}: bin packing failed for node {nloc}")
        score = caps.sum(axis=1) + (P - slots_used)
        score[~ok] = -1
        t = int(np.argmax(score))
        tile_of[nloc] = t
        slot_of[nloc] = slots_used[t]
        slots_used[t] += 1
        caps[t] -= need

    # edge stream positions
    et = tile_of[dl]
    eslot = slot_of[dl]
    o = np.lexsort((sc, et))
    et_s, sc_s, slot_s, src_s = et[o], sc[o], eslot[o], es[o]
    ks = et_s * cfg.nsc + sc_s
    counts = np.bincount(ks, minlength=cfg.tiles * cfg.nsc)
    gbase = np.concatenate([[0], np.cumsum(counts)[:-1]])
    rank = np.arange(len(ks)) - gbase[ks]

    rotpre = np.zeros((cfg.group, cfg.nsc), dtype=np.int64)
    for r in range(cfg.group):
        for s in range(cfg.nsc):
            rotpre[r, s] = sum(cfg.rot[i][s] for i in range(r))
    tt = np.arange(cfg.tiles)
    callno = (tt // cfg.group)[:, None] * cfg.nsc + np.arange(cfg.nsc)[None, :]
    pbase = callno * cfg.call_idx + rotpre[tt % cfg.group] * P  # [tiles, nsc]
    assert (counts.reshape(cfg.tiles, cfg.nsc) <= rot[tt % cfg.group] * P).all()

    total = cfg.ncalls * cfg.call_idx
    idx_flat = np.zeros(total, dtype=np.int16)
    dl_flat = np.full(total, 255.0, dtype=np.float16)
    pos = pbase[et_s, sc_s] + rank
    idx_flat[pos] = (src_s - sc_s * cfg.srcchunk).astype(np.int16)
    dl_flat[pos] = slot_s.astype(np.float16)

    ci = cfg.call_idx
    idx_w = idx_flat.reshape(cfg.ncalls, ci // 16, 16).transpose(2, 0, 1)
    idx_w = np.tile(idx_w.reshape(16, -1), (8, 1))           # [128, idxcols]
    dl_w = dl_flat.reshape(cfg.nchunks_total, P).T.copy()    # [128, nchunks]

    dcq_w = np.zeros((P, cfg.tiles), dtype=np.float32)
    dcq_w[slot_of, tile_of] = dis_c[n0:n0 + cfg.ndst]
    return idx_w, dl_w, dcq_w, tile_of, slot_of


def _prepare(cfg: Cfg, node, node_type, edge_index, embed, W1, b1, W2, b2):
    n = cfg.n
    src = edge_index[0].astype(np.int64)
    dst = edge_index[1].astype(np.int64)
    deg = (np.bincount(dst, minlength=n) + 1).astype(np.float32)
    dis = (1.0 / np.sqrt(deg.astype(np.float64))).astype(np.float32)
    s_arr = np.bincount(src, weights=dis[dst].astype(np.float64), minlength=n)
    c = (dis.astype(np.float64) * (s_arr + dis)).astype(np.float32)
    dis_c = (dis.astype(np.float64) * c).astype(np.float32)

    T8 = (embed.astype(np.float64) @ W1[F_IN:, :].astype(np.float64))
    w1e = np.zeros((FPAD, H), dtype=np.float16)
    w1e[:F_IN] = W1[:F_IN, :].astype(np.float16)
    w1e[F_IN:FEXT] = T8.astype(np.float16)

    npad2 = cfg.nsc * cfg.srcchunk
    f8np = mybir.dt.np(mybir.dt.float8e4)
    xgf = np.zeros((npad2, B, FPAD), dtype=f8np)
    xgf[:n, :, :F_IN] = (node.transpose(1, 0, 2)
                         * dis[:, None, None]).astype(f8np)
    oh8 = np.zeros((n, 8), dtype=np.float32)
    oh8[np.arange(n), node_type.astype(np.int64)] = dis
    xgf[:n, :, F_IN:FEXT] = oh8[:, None, :].astype(f8np)
    xg = xgf.reshape(npad2, B * FPAD)

    iota = np.tile(np.arange(P, dtype=np.float16), (P, 4))
    iden = np.eye(P, dtype=np.float16)

    has_b1 = bool(np.any(b1 != 0))
    in_maps = []
    for core in range(cfg.ncores):
        idx_w, dl_w, dcq_w, tile_of, slot_of = _pack_core(
            cfg, core, src, dst, dis_c, n)
        # xgo: own nodes' xg rows at (tile, slot) positions -> the self-loop
        # term enters psag via one identity matmul per tile.
        n0 = core * cfg.ndst
        xgo_w = np.zeros((cfg.tiles * P, XW), dtype=np.float16)
        xgo_w[tile_of * P + slot_of] = xg[n0:n0 + cfg.ndst].astype(np.float16)
        m = {"xg": xg, "xgo": xgo_w, "w1e": w1e, "idxt": idx_w, "dlt": dl_w,
             "dcq": dcq_w, "iot": iota, "idn": iden}
        if has_b1:
            disc_w = np.zeros((P, cfg.tiles), dtype=np.float32)
            cc_w = np.zeros((P, cfg.tiles), dtype=np.float32)
            n0 = core * cfg.ndst
            disc_w[slot_of, tile_of] = dis[n0:n0 + cfg.ndst]
            cc_w[slot_of, tile_of] = c[n0:n0 + cfg.ndst]
            m["disc"] = disc_w
            m["cct"] = cc_w
            m["b1b"] = np.tile(b1.astype(np.float32), (P, B))
        in_maps.append(m)
    return in_maps, has_b1


def run(inputs, cfg: Cfg = CFG, trace: bool = False):
    node = np.asarray(inputs["node"], dtype=np.float32)
    node_type = np.asarray(inputs["node_type"])
    edge_index = np.asarray(inputs["edge_index"])
    embed = np.asarray(inputs["embed"], dtype=np.float32)
    W1 = np.asarray(inputs["W1"], dtype=np.float32)
    b1 = np.asarray(inputs["b1"], dtype=np.float32)
    W2 = np.asarray(inputs["W2"], dtype=np.float32)
    b2 = np.asarray(inputs["b2"], dtype=np.float32)

    in_maps, has_b1 = _prepare(cfg, node, node_type, edge_index,
                               embed, W1, b1, W2, b2)
    nc = _get_program(cfg, has_b1)
    res = run_bass_kernel_spmd(
        nc, in_maps, core_ids=list(range(cfg.ncores)), trace=trace,
        trace_cores=list(range(cfg.ncores)) if trace else None)

    total = np.zeros((B, H), dtype=np.float64)
    for core in range(cfg.ncores):
        acc = res.results[core]["acc"].astype(np.float64)   # [128, 2*H]
        total += acc.reshape(P, B, H).sum(axis=0)
    out = (total @ W2.astype(np.float64)) / cfg.n + b2.astype(np.float64)
    return out.astype(np.float32), res


def kernel(**inputs) -> np.ndarray:
    # The device occasionally reports NRT_EXEC_UNIT_UNRECOVERABLE under
    # sustained load; a re-run has always recovered it, so retry once
    # (the compiled program is cached — only the execute repeats).
    try:
        out, _ = run(inputs, CFG, trace=False)
    except Exception:
        out, _ = run(inputs, CFG, trace=False)
    return out
